# revision 10
# baseline (speedup 1.0000x reference)
"""Multi-head causal attention on 8 Trainium2 NeuronCores.

Problem: B=2, S=2048, D=1024, H=16, DH=64, causal mask, f32.

Sharding: core c -> (batch b = c//4, head group g = c%4 of 4 heads).
Each core computes Q/K/V projections for its 4 heads, streamed causal
attention, then the Z tensors are AllGather'd across the 4 cores of a
batch group and each core computes a 256-column slice of the output
projection.  Host concatenates slices.

Perf structure:
 - Score matmuls are row-tiled (tile_position): the two heads of a pair
   run concurrently in the 128x128 PE array (K=64 each), halving score
   cost.
 - K/V projections are staged just-in-time into the j=3 attention
   stream; Q for j<3 and the output projections are interleaved as PE
   filler so the PE never idles (keeps HAM clock warm) while the scalar
   engine streams the softmax exps.
 - Z is AllGather'd per q-chunk (4 gathers) so only the smallest chunk's
   gather sits in the tail.
 - softmax normalization uses a fast DVE reciprocal + one DRAM-broadcast
   DMA instead of a 4-DMA round trip.
"""
import os
import numpy as np
import ml_dtypes
from contextlib import ExitStack

import concourse.bacc as bacc
import concourse.tile as tile
from concourse import mybir
from concourse import bass_utils

F32 = mybir.dt.float32
BF16 = mybir.dt.bfloat16
AF = mybir.ActivationFunctionType

B, S, D, H, DH = 2, 2048, 1024, 16, 64
NCORES = 8
HLOC = 4            # heads per core
QC = 512            # q chunk width
KT = 128            # k tile height
NKT = S // KT       # 16 k tiles
NQ = S // QC        # 4 q chunks
MC = D // 128       # 8 contraction chunks
NSL = D // 4        # 256 output columns per core
INV_SCALE = 1.0 / float(np.sqrt(DH))

_cache = {}


def _build(dbg=False):
    nc = bacc.Bacc("TRN2", target_bir_lowering=False, debug=False,
                   num_devices=NCORES)

    xT_d = nc.dram_tensor("xT", [D, S], BF16, kind="ExternalInput").ap()
    wq_d = nc.dram_tensor("wq", [D, 256], BF16, kind="ExternalInput").ap()
    wk_d = nc.dram_tensor("wk", [D, 256], BF16, kind="ExternalInput").ap()
    wv_d = nc.dram_tensor("wv", [D, 256], BF16, kind="ExternalInput").ap()
    wo_d = nc.dram_tensor("wo", [D, NSL], BF16, kind="ExternalInput").ap()
    bqk_d = nc.dram_tensor("bqk", [128, 4], F32, kind="ExternalInput").ap()
    bo_d = nc.dram_tensor("bo", [128, 2], F32, kind="ExternalInput").ap()
    triu_d = nc.dram_tensor("triu", [128, 128], BF16, kind="ExternalInput").ap()
    out_d = nc.dram_tensor("outT", [NSL, S], F32, kind="ExternalOutput").ap()
    if dbg:
        dbg_qt = nc.dram_tensor("dbg_qt", [128, 2, S], BF16, kind="ExternalOutput").ap()
        dbg_kt = nc.dram_tensor("dbg_kt", [128, HLOC, S], BF16, kind="ExternalOutput").ap()
        dbg_v = nc.dram_tensor("dbg_v", [128, NKT, 324], BF16, kind="ExternalOutput").ap()
        dbg_zf = nc.dram_tensor("dbg_zf", [16, 65, QC], F32, kind="ExternalOutput").ap()
        dbg_ri = nc.dram_tensor("dbg_ri", [16, 1, QC], F32, kind="ExternalOutput").ap()
        dbg_zt = nc.dram_tensor("dbg_zt", [NQ, HLOC * 64, QC], BF16, kind="ExternalOutput").ap()

    with tile.TileContext(nc) as tc, ExitStack() as ctx:
        singles = ctx.enter_context(tc.tile_pool(name="singles", bufs=1))
        ptpool = ctx.enter_context(tc.tile_pool(name="pt", bufs=3))
        ztpool = ctx.enter_context(tc.tile_pool(name="zt", bufs=3))
        rpool = ctx.enter_context(tc.tile_pool(name="rp", bufs=3))
        opool = ctx.enter_context(tc.tile_pool(name="op", bufs=2))
        zapool = ctx.enter_context(tc.tile_pool(name="za", bufs=2))
        ps_pool = ctx.enter_context(tc.tile_pool(name="ps", bufs=2, space="PSUM"))
        zps_pool = ctx.enter_context(tc.tile_pool(name="zps", bufs=2, space="PSUM"))
        pr_pool = ctx.enter_context(tc.tile_pool(name="pr", bufs=2, space="PSUM"))
        dram = ctx.enter_context(tc.tile_pool(name="dram", bufs=1, space="DRAM"))

        # ---------------- persistent SBUF tensors ----------------
        xt_sb = singles.tile([128, MC, S], BF16)      # x[b].T, m-chunked
        wq_sb = singles.tile([128, MC, 256], BF16)
        wk_sb = singles.tile([128, MC, 256], BF16)
        wv_sb = singles.tile([128, MC, 256], BF16)
        wo_sb = singles.tile([128, MC, NSL], BF16)
        bqk_sb = singles.tile([128, 4], F32)
        bo_sb = singles.tile([128, 2], F32)
        triu_sb = singles.tile([128, 128], BF16)
        qt_sb = singles.tile([128, 2, S], BF16)       # Q^T, head pairs packed
        # K^T per head pair: even head on partitions 0:64, odd on 64:128
        # (row-tiled score matmuls read only their own half).
        ktz_sb = singles.tile([128, HLOC, S], BF16)
        # V' layout per ktile: [V_h | 1] x 4 heads (65 cols each), padded to
        # 324 so every head can present a [128, 128] stationary slice
        v_sb = singles.tile([128, NKT, 324], BF16)
        ones_col = singles.tile([128, 1], F32)

        nc.sync.dma_start(wq_sb[:],
                          wq_d.rearrange("(c p) hd -> p c hd", p=128))
        nc.sync.dma_start(bqk_sb[:], bqk_d[:])
        xT_r = xT_d.rearrange("(c p) q -> p c q", p=128)
        for mc in range(MC):
            nc.sync.dma_start(xt_sb[:, mc, :], xT_r[:, mc, :])
        nc.sync.dma_start(wk_sb[:],
                          wk_d.rearrange("(c p) hd -> p c hd", p=128))
        nc.vector.memset(v_sb[:], 0.0)
        nc.sync.dma_start(wv_sb[:],
                          wv_d.rearrange("(c p) hd -> p c hd", p=128))
        nc.sync.dma_start(wo_sb[:],
                          wo_d.rearrange("(c p) n -> p c n", p=128))
        nc.sync.dma_start(bo_sb[:], bo_d[:])
        nc.sync.dma_start(triu_sb[:], triu_d[:])

        nc.vector.memset(ones_col[:], 1.0)
        ones_v = v_sb[:, :, 0:260].rearrange("p k (h c) -> p k h c", c=65)
        for kt in range(NKT):
            nc.vector.tensor_copy(ones_v[:, kt, :, 64],
                                  ones_col[:, 0:1].to_broadcast((128, 4)))

        # ---------------- staging + collective targets ----------------
        zt_b = [dram.tile([HLOC * 64, QC], BF16, name=f"ztb{j}")
                for j in range(NQ)]
        zt_all = [dram.tile([H * 64, QC], BF16, name=f"zta{j}")
                  for j in range(NQ)]
        r_dram = [dram.tile([1, QC], F32, name=f"rd{j}_{h}")
                  for j in range(NQ) for h in range(HLOC)]

        # ---------------- projection / oproj emission units ----------------
        def emit_Q(hp, j):
            pp = pr_pool.tile([128, QC], F32, tag="pr", name=f"q{hp}_{j}")
            jc = slice(j * QC, (j + 1) * QC)
            for mc in range(MC):
                nc.tensor.matmul(
                    pp[:], wq_sb[:, mc, hp * 128:(hp + 1) * 128],
                    xt_sb[:, mc, jc], start=(mc == 0), stop=(mc == MC - 1))
            nc.vector.tensor_scalar_add(qt_sb[:, hp, jc], pp[:],
                                        bqk_sb[:, hp:hp + 1])

        def emit_K(hp, c):
            # K for heads 2hp,2hp+1 on positions [256c, 256c+256)
            pp = pr_pool.tile([128, QC], F32, tag="pr", name=f"k{hp}_{c}")
            kc = slice(c * 256, (c + 1) * 256)
            for mc in range(MC):
                nc.tensor.matmul(
                    pp[:, 0:256], wk_sb[:, mc, hp * 128:(hp + 1) * 128],
                    xt_sb[:, mc, kc], start=(mc == 0), stop=(mc == MC - 1))
            nc.vector.tensor_scalar_add(
                ktz_sb[0:64, 2 * hp, kc], pp[0:64, 0:256],
                bqk_sb[0:64, 2 + hp:3 + hp])
            nc.vector.tensor_scalar_add(
                ktz_sb[64:128, 2 * hp + 1, kc], pp[64:128, 0:256],
                bqk_sb[64:128, 2 + hp:3 + hp])

        def emit_V(kt):
            # V[k, hd] for one ktile (no bias: folded into b_O)
            vp = pr_pool.tile([128, QC], F32, tag="pr", name=f"v{kt}")
            for mc in range(MC):
                nc.tensor.matmul(
                    vp[:, 0:256], xt_sb[:, mc, kt * 128:(kt + 1) * 128],
                    wv_sb[:, mc, :], start=(mc == 0), stop=(mc == MC - 1))
            nc.vector.tensor_copy(
                ones_v[:, kt, :, 0:64],
                vp[:, 0:256].rearrange("p (h c) -> p h c", c=64))

        oproj_ops = {}

        def oproj_unit(j, cdx):
            if cdx == 0:
                oproj_ops[j] = [pr_pool.tile([128, QC], F32, tag="pr",
                                             name=f"o{j}_{n}")
                                for n in range(2)]
            ops = oproj_ops[j]
            za = zapool.tile([128, QC], BF16, tag="za")
            nc.sync.dma_start(za[:], zt_all[j][cdx * 128:(cdx + 1) * 128, :])
            for n in range(2):
                nc.tensor.matmul(ops[n][:],
                                 wo_sb[:, cdx, n * 128:(n + 1) * 128],
                                 za[:], start=(cdx == 0), stop=(cdx == MC - 1))
            if cdx == MC - 1:
                for n in range(2):
                    ot = opool.tile([128, QC], F32, tag="ot")
                    nc.vector.tensor_scalar_add(ot[:], ops[n][:],
                                                bo_sb[:, n:n + 1])
                    nc.sync.dma_start(
                        out_d[n * 128:(n + 1) * 128, j * QC:(j + 1) * QC],
                        ot[:])
                oproj_ops.pop(j)

        # ---------------- attention ----------------
        steps = []
        for j in reversed(range(NQ)):
            for hp in range(2):
                for i in range(4 * j + 4):
                    steps.append((j, hp, i))

        sp_map = {}
        zps_cur = {}

        def emit_S(k):
            j, hp, i = steps[k]
            qq0 = max(0, i - 4 * j) * 128
            sp = ps_pool.tile([128, 2, QC], F32, tag="ps", name=f"sp{k}")
            jq = slice(j * QC + qq0, (j + 1) * QC)
            it = slice(i * 128, (i + 1) * 128)
            nc.tensor.matmul(sp[:, 0, qq0:QC], ktz_sb[0:64, 2 * hp, it],
                             qt_sb[0:64, hp, jq], start=True, stop=True,
                             tile_position=(0, 0))
            nc.tensor.matmul(sp[:, 1, qq0:QC], ktz_sb[64:128, 2 * hp + 1, it],
                             qt_sb[64:128, hp, jq], start=True, stop=True,
                             tile_position=(64, 0))
            sp_map[k] = sp

        def emit_norm(j, h, zps):
            zfull = rpool.tile([65, QC], F32, tag="zfull")
            nc.vector.tensor_copy(zfull[:], zps[0:65, :])
            # broadcast r to partitions 0:64 first (reciprocal_approx_fast
            # is a custom DVE uop that only works at base partition 0)
            rd = r_dram[j * HLOC + h]
            nc.sync.dma_start(rd[:], zfull[64:65, :])
            rb = rpool.tile([64, QC], F32, tag="rb")
            nc.sync.dma_start(rb[:], rd.to_broadcast((64, QC)))
            rbi = rpool.tile([64, QC], F32, tag="rbi")
            nc.vector.reciprocal_approx_fast(rbi[:], rb[:])
            if dbg:
                nc.sync.dma_start(dbg_zf[j * HLOC + h], zfull[:])
                nc.sync.dma_start(dbg_ri[j * HLOC + h], rbi[0:1, :])
            zt_t = ztpool.tile([64, QC], BF16, tag="zt")
            nc.vector.tensor_mul(zt_t[:], zfull[0:64, :], rbi[:])
            nc.sync.dma_start(zt_b[j][h * 64:(h + 1) * 64, :], zt_t[:])

        def emit_EZ(k):
            j, hp, i = steps[k]
            nkt_j = 4 * j + 4
            qq0 = max(0, i - 4 * j) * 128
            sp = sp_map.pop(k)
            pt = ptpool.tile([128, 2, QC], BF16, tag="pt", name=f"pt{k}")
            nc.scalar.activation(pt[:, :, qq0:], sp[:, :, qq0:], AF.Exp,
                                 bias=0.0, scale=INV_SCALE)
            t = i - 4 * j
            if t >= 0:
                for u in range(2):
                    blk = pt[:, u, 128 * t:128 * (t + 1)]
                    nc.vector.tensor_mul(blk, blk, triu_sb[:])
            if i == 0:
                zps_cur[(j, hp)] = (
                    zps_pool.tile([128, QC], F32, tag="zps", name=f"za{j}{hp}"),
                    zps_pool.tile([128, QC], F32, tag="zps", name=f"zb{j}{hp}"))
            zpair = zps_cur[(j, hp)]
            for u in range(2):
                h = 2 * hp + u
                nc.tensor.matmul(zpair[u][0:128, qq0:QC],
                                 v_sb[:, i, h * 65:h * 65 + 128],
                                 pt[:, u, qq0:QC],
                                 start=(i == 0), stop=(i == nkt_j - 1))
            if i == nkt_j - 1:
                for u in range(2):
                    emit_norm(j, 2 * hp + u, zpair[u])
                zps_cur.pop((j, hp))
                if hp == 1:
                    nc.gpsimd.collective_compute(
                        "AllGather", mybir.AluOpType.bypass,
                        replica_groups=[[0, 1, 2, 3], [4, 5, 6, 7]],
                        ins=[zt_b[j].opt()], outs=[zt_all[j].opt()])

        # filler schedule: list of callables per step index
        fillers = {k: [] for k in range(len(steps))}

        def step_index(j, hp, i):
            return steps.index((j, hp, i))

        # j=3/hp=0: stage remaining K(0,*) and V(*) (ktiles 0-3 in lead)
        q0 = []
        for c in range(2, 8):
            q0.append(("K", 0, c))
            q0.append(("V", 2 * c))
            q0.append(("V", 2 * c + 1))
        # K(1, 0..1) must land before (3,1,0)'s S (lookahead)
        q0.append(("K", 1, 0))
        q0.append(("K", 1, 1))
        base = step_index(3, 0, 0)
        for n, unit in enumerate(q0):
            fillers[base + min(15, (n * 16) // len(q0))].append(unit)
        # j=3/hp=1: rest of K(1,*), then Q for j=2,1,0
        q1 = [("K", 1, c) for c in range(2, 8)]
        q1 += [("Q", hp, j) for j in (2, 1, 0) for hp in range(2)]
        base = step_index(3, 1, 0)
        for n, unit in enumerate(q1):
            fillers[base + min(15, (n * 16) // len(q1))].append(unit)
        # oproj(3) during j=1/hp=0, oproj(2) during j=0
        for cdx in range(MC):
            fillers[step_index(1, 0, cdx)].append(("O", 3, cdx))
        for cdx in range(4):
            fillers[step_index(0, 0, cdx)].append(("O", 2, cdx))
            fillers[step_index(0, 1, cdx)].append(("O", 2, 4 + cdx))

        def run_unit(unit):
            kind = unit[0]
            if kind == "K":
                emit_K(unit[1], unit[2])
            elif kind == "V":
                emit_V(unit[1])
            elif kind == "Q":
                emit_Q(unit[1], unit[2])
            elif kind == "O":
                oproj_unit(unit[1], unit[2])

        # lead-in: Q for j=3, K/V for ktiles 0-3 (hp=0 side)
        emit_Q(0, 3)
        emit_Q(1, 3)
        emit_K(0, 0)
        emit_V(0)
        emit_V(1)
        emit_K(0, 1)
        emit_V(2)
        emit_V(3)

        emit_S(0)
        for k in range(len(steps)):
            if k + 1 < len(steps):
                emit_S(k + 1)
            for unit in fillers[k]:
                run_unit(unit)
            emit_EZ(k)

        # tail: output projections for j=1, j=0 (gathers land late)
        for j in (1, 0):
            for cdx in range(MC):
                oproj_unit(j, cdx)

        if dbg:
            nc.sync.dma_start(dbg_qt[:], qt_sb[:])
            nc.sync.dma_start(dbg_kt[:], ktz_sb[:])
            nc.sync.dma_start(dbg_v[:], v_sb[:])
            for j in range(NQ):
                nc.sync.dma_start(dbg_zt[j], zt_b[j][:])

    nc.compile()
    return nc


def _prep_inputs(x, W_Q, W_K, W_V, W_O, b_Q, b_K, b_V, b_O, mask):
    x = np.asarray(x, dtype=np.float32)
    W_Q = np.asarray(W_Q, dtype=np.float32)
    W_K = np.asarray(W_K, dtype=np.float32)
    W_V = np.asarray(W_V, dtype=np.float32)
    W_O = np.asarray(W_O, dtype=np.float32)
    b_Q = np.asarray(b_Q, dtype=np.float32)
    b_K = np.asarray(b_K, dtype=np.float32)
    b_O = np.asarray(b_O, dtype=np.float32)
    b_V = np.asarray(b_V, dtype=np.float32)
    mask = np.asarray(mask)

    # effective output bias: b_O + sum_h W_O[h] @ b_V[h]
    bo_eff = b_O + np.einsum("hnd,hd->n", W_O.astype(np.float64),
                             b_V.astype(np.float64)).astype(np.float32)
    # diagonal 128x128 block of the mask, transposed to (k, q); the kernel
    # skips all fully-masked blocks assuming causal structure
    triu = np.ascontiguousarray(mask[0:128, 0:128].T.astype(np.float32))
    # W^T packs: [m, h*64+d]
    wqT = np.ascontiguousarray(W_Q.transpose(2, 0, 1).reshape(D, H * DH))
    wkT = np.ascontiguousarray(W_K.transpose(2, 0, 1).reshape(D, H * DH))
    wvT = np.ascontiguousarray(W_V.transpose(2, 0, 1).reshape(D, H * DH))
    woT = np.ascontiguousarray(W_O.transpose(0, 2, 1).reshape(H * DH, D))

    in_maps = []
    for c in range(NCORES):
        b = c // 4
        g = c % 4
        hs = slice(4 * g * DH, 4 * (g + 1) * DH)
        bqk = np.stack([
            np.concatenate([b_Q[4 * g], b_Q[4 * g + 1]]),
            np.concatenate([b_Q[4 * g + 2], b_Q[4 * g + 3]]),
            np.concatenate([b_K[4 * g], b_K[4 * g + 1]]),
            np.concatenate([b_K[4 * g + 2], b_K[4 * g + 3]]),
        ], axis=1)
        in_maps.append({
            "xT": np.ascontiguousarray(x[b].T).astype(ml_dtypes.bfloat16),
            "wq": np.ascontiguousarray(wqT[:, hs]).astype(ml_dtypes.bfloat16),
            "wk": np.ascontiguousarray(wkT[:, hs]).astype(ml_dtypes.bfloat16),
            "wv": np.ascontiguousarray(wvT[:, hs]).astype(ml_dtypes.bfloat16),
            "wo": np.ascontiguousarray(
                woT[:, NSL * g:NSL * (g + 1)]).astype(ml_dtypes.bfloat16),
            "bqk": np.ascontiguousarray(bqk.astype(np.float32)),
            "bo": np.ascontiguousarray(
                bo_eff[NSL * g:NSL * (g + 1)].reshape(2, 128).T),
            "triu": triu.astype(ml_dtypes.bfloat16),
        })
    return in_maps


last_exec_time_ns = None


def kernel(x, W_Q, W_K, W_V, W_O, b_Q, b_K, b_V, b_O, mask):
    global last_exec_time_ns
    in_maps = _prep_inputs(x, W_Q, W_K, W_V, W_O, b_Q, b_K, b_V, b_O, mask)
    dbg = os.environ.get("KERNEL_DEBUG") == "1"
    if "nc" not in _cache:
        _cache["nc"] = _build(dbg)
    nc = _cache["nc"]

    trace = os.environ.get("KERNEL_TRACE") == "1"
    if trace:
        import sys, types
        import trn_agent_boot.trn_boot as _tb
        hook = _tb._ntff_profile_via_ctypes('/opt/axon/libaxon_pjrt.so')
        mod = types.ModuleType("antenv.axon_hooks")
        mod.get_axon_ntff_profile_hook = lambda: hook
        mod.set_axon_ntff_profile_hook = lambda h: None
        sys.modules["antenv.axon_hooks"] = mod
        bass_utils.upload_artifacts = lambda tmpdir: f"local:{tmpdir}"

    res = bass_utils.run_bass_kernel_spmd(
        nc, in_maps, core_ids=list(range(NCORES)), trace=trace)
    last_exec_time_ns = res.exec_time_ns
    _cache["last_res"] = res

    out = np.empty((B, S, D), dtype=np.float32)
    for c in range(NCORES):
        b = c // 4
        g = c % 4
        out[b, :, NSL * g:NSL * (g + 1)] = res.results[c]["outT"].T
    return out


# revision 16
# speedup vs baseline: 1.0329x; 1.0329x over previous
"""Multi-head causal attention on 8 Trainium2 NeuronCores.

Problem: B=2, S=2048, D=1024, H=16, DH=64, causal mask, f32.

Sharding: core c -> (batch b = c//4, head group g = c%4 of 4 heads).
Each core computes Q/K/V projections for its 4 heads, streamed causal
attention, then the Z tensors are AllGather'd across the 4 cores of a
batch group and each core computes a 256-column slice of the output
projection.  Host concatenates slices.

Perf structure:
 - Score matmuls are row-tiled (tile_position): the two heads of a pair
   run concurrently in the 128x128 PE array (K=64 each), halving score
   cost.
 - K/V projections are staged just-in-time into the j=3 attention
   stream; Q for j<3 and the output projections are interleaved as PE
   filler so the PE never idles (keeps HAM clock warm) while the scalar
   engine streams the softmax exps.
 - Z is AllGather'd per q-chunk (4 gathers) so only the smallest chunk's
   gather sits in the tail.
 - softmax normalization uses a fast DVE reciprocal + one DRAM-broadcast
   DMA instead of a 4-DMA round trip.
"""
import os
import numpy as np
import ml_dtypes
from contextlib import ExitStack

import concourse.bacc as bacc
import concourse.tile as tile
from concourse import mybir
from concourse import bass_utils

F32 = mybir.dt.float32
BF16 = mybir.dt.bfloat16
AF = mybir.ActivationFunctionType

B, S, D, H, DH = 2, 2048, 1024, 16, 64
NCORES = 8
HLOC = 4            # heads per core
QC = 512            # q chunk width
KT = 128            # k tile height
NKT = S // KT       # 16 k tiles
NQ = S // QC        # 4 q chunks
MC = D // 128       # 8 contraction chunks
NSL = D // 4        # 256 output columns per core
INV_SCALE = 1.0 / float(np.sqrt(DH))

_cache = {}


def _build(dbg=False):
    nc = bacc.Bacc("TRN2", target_bir_lowering=False, debug=False,
                   num_devices=NCORES)

    xT_d = nc.dram_tensor("xT", [D, S], BF16, kind="ExternalInput").ap()
    wq_d = nc.dram_tensor("wq", [D, 256], BF16, kind="ExternalInput").ap()
    wk_d = nc.dram_tensor("wk", [D, 256], BF16, kind="ExternalInput").ap()
    wv_d = nc.dram_tensor("wv", [D, 256], BF16, kind="ExternalInput").ap()
    wo_d = nc.dram_tensor("wo", [D, NSL], BF16, kind="ExternalInput").ap()
    bqk_d = nc.dram_tensor("bqk", [128, 4], F32, kind="ExternalInput").ap()
    bo_d = nc.dram_tensor("bo", [128, 2], F32, kind="ExternalInput").ap()
    triu_d = nc.dram_tensor("triu", [128, 128], BF16, kind="ExternalInput").ap()
    out_d = nc.dram_tensor("outT", [NSL, S], F32, kind="ExternalOutput").ap()
    if dbg:
        dbg_qt = nc.dram_tensor("dbg_qt", [128, 2, S], BF16, kind="ExternalOutput").ap()
        dbg_kt = nc.dram_tensor("dbg_kt", [128, HLOC, S], BF16, kind="ExternalOutput").ap()
        dbg_v = nc.dram_tensor("dbg_v", [128, NKT, 324], BF16, kind="ExternalOutput").ap()
        dbg_zf = nc.dram_tensor("dbg_zf", [16, 65, QC], F32, kind="ExternalOutput").ap()
        dbg_ri = nc.dram_tensor("dbg_ri", [16, 1, QC], F32, kind="ExternalOutput").ap()
        dbg_zt = nc.dram_tensor("dbg_zt", [NQ, HLOC * 64, QC], BF16, kind="ExternalOutput").ap()

    with tile.TileContext(nc) as tc, ExitStack() as ctx:
        singles = ctx.enter_context(tc.tile_pool(name="singles", bufs=1))
        ptpool = ctx.enter_context(tc.tile_pool(name="pt", bufs=3))
        ztpool = ctx.enter_context(tc.tile_pool(name="zt", bufs=3))
        rpool = ctx.enter_context(tc.tile_pool(name="rp", bufs=3))
        opool = ctx.enter_context(tc.tile_pool(name="op", bufs=2))
        zapool = ctx.enter_context(tc.tile_pool(name="za", bufs=2))
        ps_pool = ctx.enter_context(tc.tile_pool(name="ps", bufs=2, space="PSUM"))
        zps_pool = ctx.enter_context(tc.tile_pool(name="zps", bufs=2, space="PSUM"))
        pr_pool = ctx.enter_context(tc.tile_pool(name="pr", bufs=2, space="PSUM"))
        dram = ctx.enter_context(tc.tile_pool(name="dram", bufs=1, space="DRAM"))

        # ---------------- persistent SBUF tensors ----------------
        xt_sb = singles.tile([128, MC, S], BF16)      # x[b].T, m-chunked
        wq_sb = singles.tile([128, MC, 256], BF16)
        wk_sb = singles.tile([128, MC, 256], BF16)
        wv_sb = singles.tile([128, MC, 256], BF16)
        wo_sb = singles.tile([128, MC, NSL], BF16)
        bqk_sb = singles.tile([128, 4], F32)
        bo_sb = singles.tile([128, 2], F32)
        triu_sb = singles.tile([128, 128], BF16)
        qt_sb = singles.tile([128, 2, S], BF16)       # Q^T, head pairs packed
        # K^T per head pair: even head on partitions 0:64, odd on 64:128
        # (row-tiled score matmuls read only their own half).
        ktz_sb = singles.tile([128, HLOC, S], BF16)
        # V' layout per ktile: [V_h | 1] x 4 heads (65 cols each), padded to
        # 324 so every head can present a [128, 128] stationary slice
        v_sb = singles.tile([128, NKT, 324], BF16)
        ones_col = singles.tile([128, 1], F32)

        nc.sync.dma_start(wq_sb[:],
                          wq_d.rearrange("(c p) hd -> p c hd", p=128))
        nc.sync.dma_start(bqk_sb[:], bqk_d[:])
        xT_r = xT_d.rearrange("(c p) q -> p c q", p=128)
        for mc in range(MC):
            nc.sync.dma_start(xt_sb[:, mc, :], xT_r[:, mc, :])
        nc.sync.dma_start(wk_sb[:],
                          wk_d.rearrange("(c p) hd -> p c hd", p=128))
        nc.vector.memset(v_sb[:], 0.0)
        nc.sync.dma_start(wv_sb[:],
                          wv_d.rearrange("(c p) hd -> p c hd", p=128))
        nc.sync.dma_start(wo_sb[:],
                          wo_d.rearrange("(c p) n -> p c n", p=128))
        nc.sync.dma_start(bo_sb[:], bo_d[:])
        nc.sync.dma_start(triu_sb[:], triu_d[:])

        nc.vector.memset(ones_col[:], 1.0)
        ones_v = v_sb[:, :, 0:260].rearrange("p k (h c) -> p k h c", c=65)
        for kt in range(NKT):
            nc.vector.tensor_copy(ones_v[:, kt, :, 64],
                                  ones_col[:, 0:1].to_broadcast((128, 4)))

        # ---------------- staging + collective targets ----------------
        # Z gathered per (q-chunk, head-pair): 8 small AllGathers spread
        # through the attention phase instead of 4 big ones at the end.
        zt_b = [[dram.tile([128, QC], BF16, name=f"ztb{j}_{hp}")
                 for hp in range(2)] for j in range(NQ)]
        zt_all = [[dram.tile([4 * 128, QC], BF16, name=f"zta{j}_{hp}")
                   for hp in range(2)] for j in range(NQ)]
        r_dram = [dram.tile([1, QC], F32, name=f"rd{j}_{h}")
                  for j in range(NQ) for h in range(HLOC)]

        # ---------------- projection / oproj emission units ----------------
        def emit_Q(hp, j):
            pp = pr_pool.tile([128, QC], F32, tag="pr", name=f"q{hp}_{j}")
            jc = slice(j * QC, (j + 1) * QC)
            for mc in range(MC):
                nc.tensor.matmul(
                    pp[:], wq_sb[:, mc, hp * 128:(hp + 1) * 128],
                    xt_sb[:, mc, jc], start=(mc == 0), stop=(mc == MC - 1))
            nc.vector.tensor_scalar_add(qt_sb[:, hp, jc], pp[:],
                                        bqk_sb[:, hp:hp + 1])

        def emit_K(hp, c):
            # K for heads 2hp,2hp+1 on positions [256c, 256c+256)
            pp = pr_pool.tile([128, QC], F32, tag="pr", name=f"k{hp}_{c}")
            kc = slice(c * 256, (c + 1) * 256)
            for mc in range(MC):
                nc.tensor.matmul(
                    pp[:, 0:256], wk_sb[:, mc, hp * 128:(hp + 1) * 128],
                    xt_sb[:, mc, kc], start=(mc == 0), stop=(mc == MC - 1))
            nc.vector.tensor_scalar_add(
                ktz_sb[0:64, 2 * hp, kc], pp[0:64, 0:256],
                bqk_sb[0:64, 2 + hp:3 + hp])
            nc.vector.tensor_scalar_add(
                ktz_sb[64:128, 2 * hp + 1, kc], pp[64:128, 0:256],
                bqk_sb[64:128, 2 + hp:3 + hp])

        def emit_V(kt):
            # V[k, hd] for one ktile (no bias: folded into b_O)
            vp = pr_pool.tile([128, QC], F32, tag="pr", name=f"v{kt}")
            for mc in range(MC):
                nc.tensor.matmul(
                    vp[:, 0:256], xt_sb[:, mc, kt * 128:(kt + 1) * 128],
                    wv_sb[:, mc, :], start=(mc == 0), stop=(mc == MC - 1))
            nc.vector.tensor_copy(
                ones_v[:, kt, :, 0:64],
                vp[:, 0:256].rearrange("p (h c) -> p h c", c=64))

        oproj_ops = {}

        def oproj_unit(j, cdx):
            if cdx == 0:
                oproj_ops[j] = [pr_pool.tile([128, QC], F32, tag="pr",
                                             name=f"o{j}_{n}")
                                for n in range(2)]
            ops = oproj_ops[j]
            za = zapool.tile([128, QC], BF16, tag="za")
            # global head pair cdx = rank cdx//2, local head-pair cdx%2
            nc.sync.dma_start(
                za[:],
                zt_all[j][cdx % 2][(cdx // 2) * 128:(cdx // 2 + 1) * 128, :])
            for n in range(2):
                nc.tensor.matmul(ops[n][:],
                                 wo_sb[:, cdx, n * 128:(n + 1) * 128],
                                 za[:], start=(cdx == 0), stop=(cdx == MC - 1))
            if cdx == MC - 1:
                for n in range(2):
                    ot = opool.tile([128, QC], F32, tag="ot")
                    nc.vector.tensor_scalar_add(ot[:], ops[n][:],
                                                bo_sb[:, n:n + 1])
                    nc.sync.dma_start(
                        out_d[n * 128:(n + 1) * 128, j * QC:(j + 1) * QC],
                        ot[:])
                oproj_ops.pop(j)

        # ---------------- attention ----------------
        steps = []
        for j in reversed(range(NQ)):
            for hp in range(2):
                for i in range(4 * j + 4):
                    steps.append((j, hp, i))

        sp_map = {}
        zps_cur = {}

        def emit_S(k):
            j, hp, i = steps[k]
            qq0 = max(0, i - 4 * j) * 128
            sp = ps_pool.tile([128, 2, QC], F32, tag="ps", name=f"sp{k}")
            jq = slice(j * QC + qq0, (j + 1) * QC)
            it = slice(i * 128, (i + 1) * 128)
            nc.tensor.matmul(sp[:, 0, qq0:QC], ktz_sb[0:64, 2 * hp, it],
                             qt_sb[0:64, hp, jq], start=True, stop=True,
                             tile_position=(0, 0))
            nc.tensor.matmul(sp[:, 1, qq0:QC], ktz_sb[64:128, 2 * hp + 1, it],
                             qt_sb[64:128, hp, jq], start=True, stop=True,
                             tile_position=(64, 0))
            sp_map[k] = sp

        def emit_norm(j, h, zps):
            zfull = rpool.tile([65, QC], F32, tag="zfull")
            nc.vector.tensor_copy(zfull[:], zps[0:65, :])
            # broadcast r to partitions 0:64 first (reciprocal_approx_fast
            # is a custom DVE uop that only works at base partition 0)
            rd = r_dram[j * HLOC + h]
            nc.sync.dma_start(rd[:], zfull[64:65, :])
            rb = rpool.tile([64, QC], F32, tag="rb")
            nc.sync.dma_start(rb[:], rd.to_broadcast((64, QC)))
            rbi = rpool.tile([64, QC], F32, tag="rbi")
            nc.vector.reciprocal_approx_fast(rbi[:], rb[:])
            if dbg:
                nc.sync.dma_start(dbg_zf[j * HLOC + h], zfull[:])
                nc.sync.dma_start(dbg_ri[j * HLOC + h], rbi[0:1, :])
            zt_t = ztpool.tile([64, QC], BF16, tag="zt")
            nc.vector.tensor_mul(zt_t[:], zfull[0:64, :], rbi[:])
            nc.sync.dma_start(
                zt_b[j][h // 2][(h % 2) * 64:(h % 2 + 1) * 64, :], zt_t[:])

        def emit_EZ(k):
            j, hp, i = steps[k]
            nkt_j = 4 * j + 4
            qq0 = max(0, i - 4 * j) * 128
            sp = sp_map.pop(k)
            pt = ptpool.tile([128, 2, QC], BF16, tag="pt", name=f"pt{k}")
            nc.scalar.activation(pt[:, :, qq0:], sp[:, :, qq0:], AF.Exp,
                                 bias=0.0, scale=INV_SCALE)
            t = i - 4 * j
            if t >= 0:
                for u in range(2):
                    blk = pt[:, u, 128 * t:128 * (t + 1)]
                    nc.vector.tensor_mul(blk, blk, triu_sb[:])
            if i == 0:
                zps_cur[(j, hp)] = (
                    zps_pool.tile([128, QC], F32, tag="zps", name=f"za{j}{hp}"),
                    zps_pool.tile([128, QC], F32, tag="zps", name=f"zb{j}{hp}"))
            zpair = zps_cur[(j, hp)]
            for u in range(2):
                h = 2 * hp + u
                nc.tensor.matmul(zpair[u][0:128, qq0:QC],
                                 v_sb[:, i, h * 65:h * 65 + 128],
                                 pt[:, u, qq0:QC],
                                 start=(i == 0), stop=(i == nkt_j - 1))
            if i == nkt_j - 1:
                for u in range(2):
                    emit_norm(j, 2 * hp + u, zpair[u])
                zps_cur.pop((j, hp))
                nc.gpsimd.collective_compute(
                    "AllGather", mybir.AluOpType.bypass,
                    replica_groups=[[0, 1, 2, 3], [4, 5, 6, 7]],
                    ins=[zt_b[j][hp].opt()], outs=[zt_all[j][hp].opt()])

        # filler schedule: list of callables per step index
        fillers = {k: [] for k in range(len(steps))}

        def step_index(j, hp, i):
            return steps.index((j, hp, i))

        # j=3/hp=0: stage remaining K(0,*) and V(*) (ktiles 0-3 in lead)
        q0 = []
        for c in range(2, 8):
            q0.append(("K", 0, c))
            q0.append(("V", 2 * c))
            q0.append(("V", 2 * c + 1))
        # K(1, 0..1) must land before (3,1,0)'s S (lookahead)
        q0.append(("K", 1, 0))
        q0.append(("K", 1, 1))
        base = step_index(3, 0, 0)
        for n, unit in enumerate(q0):
            fillers[base + min(15, (n * 16) // len(q0))].append(unit)
        # j=3/hp=1: rest of K(1,*), then Q for j=2,1,0
        q1 = [("K", 1, c) for c in range(2, 8)]
        q1 += [("Q", hp, j) for j in (2, 1, 0) for hp in range(2)]
        base = step_index(3, 1, 0)
        for n, unit in enumerate(q1):
            fillers[base + min(15, (n * 16) // len(q1))].append(unit)
        # oproj(3) during j=0 (its gathers land early in the j=1 phase)
        for cdx in range(4):
            fillers[step_index(0, 0, cdx)].append(("O", 3, cdx))
            fillers[step_index(0, 1, cdx)].append(("O", 3, 4 + cdx))

        def run_unit(unit):
            kind = unit[0]
            if kind == "K":
                emit_K(unit[1], unit[2])
            elif kind == "V":
                emit_V(unit[1])
            elif kind == "Q":
                emit_Q(unit[1], unit[2])
            elif kind == "O":
                oproj_unit(unit[1], unit[2])

        # lead-in: Q for j=3, K/V for ktiles 0-3 (hp=0 side)
        emit_Q(0, 3)
        emit_Q(1, 3)
        emit_K(0, 0)
        emit_V(0)
        emit_V(1)
        emit_K(0, 1)
        emit_V(2)
        emit_V(3)

        emit_S(0)
        for k in range(len(steps)):
            if k + 1 < len(steps):
                emit_S(k + 1)
            for unit in fillers[k]:
                run_unit(unit)
            emit_EZ(k)

        # tail: remaining output projections (gathers mostly landed already)
        for j in (2, 1, 0):
            for cdx in range(MC):
                oproj_unit(j, cdx)

        if dbg:
            nc.sync.dma_start(dbg_qt[:], qt_sb[:])
            nc.sync.dma_start(dbg_kt[:], ktz_sb[:])
            nc.sync.dma_start(dbg_v[:], v_sb[:])

    nc.compile()
    return nc


def _prep_inputs(x, W_Q, W_K, W_V, W_O, b_Q, b_K, b_V, b_O, mask):
    x = np.asarray(x, dtype=np.float32)
    W_Q = np.asarray(W_Q, dtype=np.float32)
    W_K = np.asarray(W_K, dtype=np.float32)
    W_V = np.asarray(W_V, dtype=np.float32)
    W_O = np.asarray(W_O, dtype=np.float32)
    b_Q = np.asarray(b_Q, dtype=np.float32)
    b_K = np.asarray(b_K, dtype=np.float32)
    b_O = np.asarray(b_O, dtype=np.float32)
    b_V = np.asarray(b_V, dtype=np.float32)
    mask = np.asarray(mask)

    # effective output bias: b_O + sum_h W_O[h] @ b_V[h]
    bo_eff = b_O + np.einsum("hnd,hd->n", W_O.astype(np.float64),
                             b_V.astype(np.float64)).astype(np.float32)
    # diagonal 128x128 block of the mask, transposed to (k, q); the kernel
    # skips all fully-masked blocks assuming causal structure
    triu = np.ascontiguousarray(mask[0:128, 0:128].T.astype(np.float32))
    # W^T packs: [m, h*64+d]
    wqT = np.ascontiguousarray(W_Q.transpose(2, 0, 1).reshape(D, H * DH))
    wkT = np.ascontiguousarray(W_K.transpose(2, 0, 1).reshape(D, H * DH))
    wvT = np.ascontiguousarray(W_V.transpose(2, 0, 1).reshape(D, H * DH))
    woT = np.ascontiguousarray(W_O.transpose(0, 2, 1).reshape(H * DH, D))

    in_maps = []
    for c in range(NCORES):
        b = c // 4
        g = c % 4
        hs = slice(4 * g * DH, 4 * (g + 1) * DH)
        bqk = np.stack([
            np.concatenate([b_Q[4 * g], b_Q[4 * g + 1]]),
            np.concatenate([b_Q[4 * g + 2], b_Q[4 * g + 3]]),
            np.concatenate([b_K[4 * g], b_K[4 * g + 1]]),
            np.concatenate([b_K[4 * g + 2], b_K[4 * g + 3]]),
        ], axis=1)
        in_maps.append({
            "xT": np.ascontiguousarray(x[b].T).astype(ml_dtypes.bfloat16),
            "wq": np.ascontiguousarray(wqT[:, hs]).astype(ml_dtypes.bfloat16),
            "wk": np.ascontiguousarray(wkT[:, hs]).astype(ml_dtypes.bfloat16),
            "wv": np.ascontiguousarray(wvT[:, hs]).astype(ml_dtypes.bfloat16),
            "wo": np.ascontiguousarray(
                woT[:, NSL * g:NSL * (g + 1)]).astype(ml_dtypes.bfloat16),
            "bqk": np.ascontiguousarray(bqk.astype(np.float32)),
            "bo": np.ascontiguousarray(
                bo_eff[NSL * g:NSL * (g + 1)].reshape(2, 128).T),
            "triu": triu.astype(ml_dtypes.bfloat16),
        })
    return in_maps


last_exec_time_ns = None


def kernel(x, W_Q, W_K, W_V, W_O, b_Q, b_K, b_V, b_O, mask):
    global last_exec_time_ns
    in_maps = _prep_inputs(x, W_Q, W_K, W_V, W_O, b_Q, b_K, b_V, b_O, mask)
    dbg = os.environ.get("KERNEL_DEBUG") == "1"
    if "nc" not in _cache:
        _cache["nc"] = _build(dbg)
    nc = _cache["nc"]

    trace = os.environ.get("KERNEL_TRACE") == "1"
    if trace:
        import sys, types
        import trn_agent_boot.trn_boot as _tb
        hook = _tb._ntff_profile_via_ctypes('/opt/axon/libaxon_pjrt.so')
        mod = types.ModuleType("antenv.axon_hooks")
        mod.get_axon_ntff_profile_hook = lambda: hook
        mod.set_axon_ntff_profile_hook = lambda h: None
        sys.modules["antenv.axon_hooks"] = mod
        bass_utils.upload_artifacts = lambda tmpdir: f"local:{tmpdir}"

    res = bass_utils.run_bass_kernel_spmd(
        nc, in_maps, core_ids=list(range(NCORES)), trace=trace)
    last_exec_time_ns = res.exec_time_ns
    _cache["last_res"] = res

    out = np.empty((B, S, D), dtype=np.float32)
    for c in range(NCORES):
        b = c // 4
        g = c % 4
        out[b, :, NSL * g:NSL * (g + 1)] = res.results[c]["outT"].T
    return out


# revision 21
# speedup vs baseline: 1.2800x; 1.2392x over previous
"""Multi-head causal attention on 8 Trainium2 NeuronCores.

Problem: B=2, S=2048, D=1024, H=16, DH=64, causal mask, f32.

Sharding: core c -> (batch b = c//4, head group g = c%4 of 4 heads).
Each core computes Q/K/V projections for its 4 heads, streamed causal
attention, then the Z tensors are AllGather'd across the 4 cores of a
batch group and each core computes a 256-column slice of the output
projection.  Host concatenates slices.

Perf structure:
 - Score matmuls are row-tiled (tile_position): the two heads of a pair
   run concurrently in the 128x128 PE array (K=64 each), halving score
   cost.
 - K/V projections are staged just-in-time into the j=3 attention
   stream; Q for j<3 and the output projections are interleaved as PE
   filler so the PE never idles (keeps HAM clock warm) while the scalar
   engine streams the softmax exps.
 - Z is AllGather'd per q-chunk (4 gathers) so only the smallest chunk's
   gather sits in the tail.
 - softmax normalization uses a fast DVE reciprocal + one DRAM-broadcast
   DMA instead of a 4-DMA round trip.
"""
import os
import numpy as np
import ml_dtypes
from contextlib import ExitStack

import concourse.bacc as bacc
import concourse.tile as tile
from concourse import mybir
from concourse import bass_utils

F32 = mybir.dt.float32
BF16 = mybir.dt.bfloat16
AF = mybir.ActivationFunctionType

B, S, D, H, DH = 2, 2048, 1024, 16, 64
NCORES = 8
HLOC = 4            # heads per core
QC = 512            # q chunk width
KT = 128            # k tile height
NKT = S // KT       # 16 k tiles
NQ = S // QC        # 4 q chunks
MC = D // 128       # 8 contraction chunks
NSL = D // 4        # 256 output columns per core
INV_SCALE = 1.0 / float(np.sqrt(DH))

_cache = {}


def _build(dbg=False):
    nc = bacc.Bacc("TRN2", target_bir_lowering=False, debug=False,
                   num_devices=NCORES)

    xT_d = nc.dram_tensor("xT", [D, S], BF16, kind="ExternalInput").ap()
    wq_d = nc.dram_tensor("wq", [D, 256], BF16, kind="ExternalInput").ap()
    wk_d = nc.dram_tensor("wk", [D, 256], BF16, kind="ExternalInput").ap()
    wv_d = nc.dram_tensor("wv", [D, 256], BF16, kind="ExternalInput").ap()
    wo_d = nc.dram_tensor("wo", [D, NSL], BF16, kind="ExternalInput").ap()
    bqk_d = nc.dram_tensor("bqk", [128, 4], F32, kind="ExternalInput").ap()
    bo_d = nc.dram_tensor("bo", [128, 2], F32, kind="ExternalInput").ap()
    triu_d = nc.dram_tensor("triu", [128, 128], BF16, kind="ExternalInput").ap()
    out_d = nc.dram_tensor("outT", [NSL, S], F32, kind="ExternalOutput").ap()
    if dbg:
        dbg_qt = nc.dram_tensor("dbg_qt", [128, 2, S], BF16, kind="ExternalOutput").ap()
        dbg_kt = nc.dram_tensor("dbg_kt", [128, HLOC, S], BF16, kind="ExternalOutput").ap()
        dbg_v = nc.dram_tensor("dbg_v", [128, NKT, 324], BF16, kind="ExternalOutput").ap()
        dbg_zf = nc.dram_tensor("dbg_zf", [16, 65, QC], F32, kind="ExternalOutput").ap()
        dbg_ri = nc.dram_tensor("dbg_ri", [16, 1, QC], F32, kind="ExternalOutput").ap()
        dbg_zt = nc.dram_tensor("dbg_zt", [NQ, HLOC * 64, QC], BF16, kind="ExternalOutput").ap()

    with tile.TileContext(nc) as tc, ExitStack() as ctx:
        singles = ctx.enter_context(tc.tile_pool(name="singles", bufs=1))
        ptpool = ctx.enter_context(tc.tile_pool(name="pt", bufs=3))
        ztpool = ctx.enter_context(tc.tile_pool(name="zt", bufs=3))
        rpool = ctx.enter_context(tc.tile_pool(name="rp", bufs=3))
        opool = ctx.enter_context(tc.tile_pool(name="op", bufs=2))
        zapool = ctx.enter_context(tc.tile_pool(name="za", bufs=2))
        ps_pool = ctx.enter_context(tc.tile_pool(name="ps", bufs=2, space="PSUM"))
        zps_pool = ctx.enter_context(tc.tile_pool(name="zps", bufs=2, space="PSUM"))
        pr_pool = ctx.enter_context(tc.tile_pool(name="pr", bufs=2, space="PSUM"))
        dram = ctx.enter_context(tc.tile_pool(name="dram", bufs=1, space="DRAM"))

        # ---------------- persistent SBUF tensors ----------------
        xt_sb = singles.tile([128, MC, S], BF16)      # x[b].T, m-chunked
        wq_sb = singles.tile([128, MC, 256], BF16)
        wk_sb = singles.tile([128, MC, 256], BF16)
        wv_sb = singles.tile([128, MC, 256], BF16)
        wo_sb = singles.tile([128, MC, NSL], BF16)
        bqk_sb = singles.tile([128, 4], F32)
        bo_sb = singles.tile([128, 2], F32)
        triu_sb = singles.tile([128, 128], BF16)
        qt_sb = singles.tile([128, 2, S], BF16)       # Q^T, head pairs packed
        # K^T per head pair: even head on partitions 0:64, odd on 64:128
        # (row-tiled score matmuls read only their own half).
        ktz_sb = singles.tile([128, HLOC, S], BF16)
        # V' layout per ktile: [V_h | 1] x 4 heads (65 cols each), padded to
        # 324 so every head can present a [128, 128] stationary slice
        v_sb = singles.tile([128, NKT, 324], BF16)
        ones_col = singles.tile([128, 1], F32)

        nc.sync.dma_start(wq_sb[:],
                          wq_d.rearrange("(c p) hd -> p c hd", p=128))
        nc.sync.dma_start(bqk_sb[:], bqk_d[:])
        xT_r = xT_d.rearrange("(c p) q -> p c q", p=128)
        # front half of x first: unblocks Q(j0/j1), K/V ktiles 0-7 early
        for mc in range(MC):
            nc.sync.dma_start(xt_sb[:, mc, 0:1024], xT_r[:, mc, 0:1024])
        nc.sync.dma_start(wk_sb[:],
                          wk_d.rearrange("(c p) hd -> p c hd", p=128))
        nc.vector.memset(v_sb[:], 0.0)
        nc.sync.dma_start(wv_sb[:],
                          wv_d.rearrange("(c p) hd -> p c hd", p=128))
        for mc in range(MC):
            nc.sync.dma_start(xt_sb[:, mc, 1024:2048], xT_r[:, mc, 1024:2048])
        nc.sync.dma_start(wo_sb[:],
                          wo_d.rearrange("(c p) n -> p c n", p=128))
        nc.sync.dma_start(bo_sb[:], bo_d[:])
        nc.sync.dma_start(triu_sb[:], triu_d[:])

        nc.vector.memset(ones_col[:], 1.0)
        ones_v = v_sb[:, :, 0:260].rearrange("p k (h c) -> p k h c", c=65)
        for kt in range(NKT):
            nc.vector.tensor_copy(ones_v[:, kt, :, 64],
                                  ones_col[:, 0:1].to_broadcast((128, 4)))

        # ---------------- staging + collective targets ----------------
        # Z gathered per (q-chunk, head-pair): 8 small AllGathers spread
        # through the attention phase instead of 4 big ones at the end.
        zt_b = [[dram.tile([128, QC], BF16, name=f"ztb{j}_{hp}")
                 for hp in range(2)] for j in range(NQ)]
        zt_all = [[dram.tile([4 * 128, QC], BF16, name=f"zta{j}_{hp}")
                   for hp in range(2)] for j in range(NQ)]
        r_dram = [dram.tile([1, QC], F32, name=f"rd{j}_{h}")
                  for j in range(NQ) for h in range(HLOC)]

        # ---------------- projection / oproj emission units ----------------
        def emit_Q(hp, j):
            pp = pr_pool.tile([128, QC], F32, tag="pr", name=f"q{hp}_{j}")
            jc = slice(j * QC, (j + 1) * QC)
            for mc in range(MC):
                nc.tensor.matmul(
                    pp[:], wq_sb[:, mc, hp * 128:(hp + 1) * 128],
                    xt_sb[:, mc, jc], start=(mc == 0), stop=(mc == MC - 1))
            nc.vector.tensor_scalar_add(qt_sb[:, hp, jc], pp[:],
                                        bqk_sb[:, hp:hp + 1])

        def emit_K(hp, c):
            # K for heads 2hp,2hp+1 on positions [256c, 256c+256)
            pp = pr_pool.tile([128, QC], F32, tag="pr", name=f"k{hp}_{c}")
            kc = slice(c * 256, (c + 1) * 256)
            for mc in range(MC):
                nc.tensor.matmul(
                    pp[:, 0:256], wk_sb[:, mc, hp * 128:(hp + 1) * 128],
                    xt_sb[:, mc, kc], start=(mc == 0), stop=(mc == MC - 1))
            nc.vector.tensor_scalar_add(
                ktz_sb[0:64, 2 * hp, kc], pp[0:64, 0:256],
                bqk_sb[0:64, 2 + hp:3 + hp])
            nc.vector.tensor_scalar_add(
                ktz_sb[64:128, 2 * hp + 1, kc], pp[64:128, 0:256],
                bqk_sb[64:128, 2 + hp:3 + hp])

        def emit_V(kt):
            # V[k, hd] for one ktile (no bias: folded into b_O)
            vp = pr_pool.tile([128, QC], F32, tag="pr", name=f"v{kt}")
            for mc in range(MC):
                nc.tensor.matmul(
                    vp[:, 0:256], xt_sb[:, mc, kt * 128:(kt + 1) * 128],
                    wv_sb[:, mc, :], start=(mc == 0), stop=(mc == MC - 1))
            nc.vector.tensor_copy(
                ones_v[:, kt, :, 0:64],
                vp[:, 0:256].rearrange("p (h c) -> p h c", c=64))

        oproj_ops = {}

        def oproj_unit(j, cdx):
            if cdx == 0:
                oproj_ops[j] = [pr_pool.tile([128, QC], F32, tag="pr",
                                             name=f"o{j}_{n}")
                                for n in range(2)]
            ops = oproj_ops[j]
            za = zapool.tile([128, QC], BF16, tag="za")
            # global head pair cdx = rank cdx//2, local head-pair cdx%2
            nc.sync.dma_start(
                za[:],
                zt_all[j][cdx % 2][(cdx // 2) * 128:(cdx // 2 + 1) * 128, :])
            for n in range(2):
                nc.tensor.matmul(ops[n][:],
                                 wo_sb[:, cdx, n * 128:(n + 1) * 128],
                                 za[:], start=(cdx == 0), stop=(cdx == MC - 1))
            if cdx == MC - 1:
                for n in range(2):
                    ot = opool.tile([128, QC], F32, tag="ot")
                    nc.vector.tensor_scalar_add(ot[:], ops[n][:],
                                                bo_sb[:, n:n + 1])
                    nc.sync.dma_start(
                        out_d[n * 128:(n + 1) * 128, j * QC:(j + 1) * QC],
                        ot[:])
                oproj_ops.pop(j)

        # ---------------- attention ----------------
        # ascending q-chunks: small chunks complete first so their Z
        # AllGathers start ~35us in and overlap the rest of attention
        steps = []
        for j in range(NQ):
            for hp in range(2):
                for i in range(4 * j + 4):
                    steps.append((j, hp, i))

        sp_map = {}
        zps_cur = {}

        def emit_S(k):
            j, hp, i = steps[k]
            qq0 = max(0, i - 4 * j) * 128
            sp = ps_pool.tile([128, 2, QC], F32, tag="ps", name=f"sp{k}")
            jq = slice(j * QC + qq0, (j + 1) * QC)
            it = slice(i * 128, (i + 1) * 128)
            nc.tensor.matmul(sp[:, 0, qq0:QC], ktz_sb[0:64, 2 * hp, it],
                             qt_sb[0:64, hp, jq], start=True, stop=True,
                             tile_position=(0, 0))
            nc.tensor.matmul(sp[:, 1, qq0:QC], ktz_sb[64:128, 2 * hp + 1, it],
                             qt_sb[64:128, hp, jq], start=True, stop=True,
                             tile_position=(64, 0))
            sp_map[k] = sp

        def emit_norm(j, h, zps):
            zfull = rpool.tile([65, QC], F32, tag="zfull")
            nc.vector.tensor_copy(zfull[:], zps[0:65, :])
            # broadcast r to partitions 0:64 first (reciprocal_approx_fast
            # is a custom DVE uop that only works at base partition 0)
            rd = r_dram[j * HLOC + h]
            nc.sync.dma_start(rd[:], zfull[64:65, :])
            rb = rpool.tile([64, QC], F32, tag="rb")
            nc.sync.dma_start(rb[:], rd.to_broadcast((64, QC)))
            rbi = rpool.tile([64, QC], F32, tag="rbi")
            nc.vector.reciprocal_approx_fast(rbi[:], rb[:])
            if dbg:
                nc.sync.dma_start(dbg_zf[j * HLOC + h], zfull[:])
                nc.sync.dma_start(dbg_ri[j * HLOC + h], rbi[0:1, :])
            zt_t = ztpool.tile([64, QC], BF16, tag="zt")
            nc.vector.tensor_mul(zt_t[:], zfull[0:64, :], rbi[:])
            nc.sync.dma_start(
                zt_b[j][h // 2][(h % 2) * 64:(h % 2 + 1) * 64, :], zt_t[:])

        def emit_EZ(k):
            j, hp, i = steps[k]
            nkt_j = 4 * j + 4
            qq0 = max(0, i - 4 * j) * 128
            sp = sp_map.pop(k)
            pt = ptpool.tile([128, 2, QC], BF16, tag="pt", name=f"pt{k}")
            nc.scalar.activation(pt[:, :, qq0:], sp[:, :, qq0:], AF.Exp,
                                 bias=0.0, scale=INV_SCALE)
            t = i - 4 * j
            if t >= 0:
                for u in range(2):
                    blk = pt[:, u, 128 * t:128 * (t + 1)]
                    nc.vector.tensor_mul(blk, blk, triu_sb[:])
            if i == 0:
                zps_cur[(j, hp)] = (
                    zps_pool.tile([128, QC], F32, tag="zps", name=f"za{j}{hp}"),
                    zps_pool.tile([128, QC], F32, tag="zps", name=f"zb{j}{hp}"))
            zpair = zps_cur[(j, hp)]
            for u in range(2):
                h = 2 * hp + u
                nc.tensor.matmul(zpair[u][0:128, qq0:QC],
                                 v_sb[:, i, h * 65:h * 65 + 128],
                                 pt[:, u, qq0:QC],
                                 start=(i == 0), stop=(i == nkt_j - 1))
            if i == nkt_j - 1:
                for u in range(2):
                    emit_norm(j, 2 * hp + u, zpair[u])
                zps_cur.pop((j, hp))
                nc.gpsimd.collective_compute(
                    "AllGather", mybir.AluOpType.bypass,
                    replica_groups=[[0, 1, 2, 3], [4, 5, 6, 7]],
                    ins=[zt_b[j][hp].opt()], outs=[zt_all[j][hp].opt()])

        # filler schedule: list of callables per step index
        fillers = {k: [] for k in range(len(steps))}

        def step_index(j, hp, i):
            return steps.index((j, hp, i))

        def place(j, hp, i, *unit):
            fillers[step_index(j, hp, i)].append(unit)

        # stage K/V/Q just ahead of first use, Oproj after gathers land
        place(0, 0, 0, "K", 1, 0)
        place(0, 0, 1, "K", 1, 1)
        place(0, 0, 2, "Q", 0, 1)
        place(0, 0, 3, "Q", 1, 1)
        place(0, 1, 0, "K", 0, 2)
        place(0, 1, 1, "V", 4)
        place(0, 1, 2, "V", 5)
        place(0, 1, 3, "K", 1, 2)
        for n, u in enumerate([("V", 6), ("V", 7), ("K", 0, 3), ("K", 1, 3),
                               ("Q", 0, 2), ("Q", 1, 2), ("K", 0, 4),
                               ("V", 8)]):
            place(1, 0, n, *u)
        for n, u in enumerate([("V", 9), ("K", 0, 5), ("V", 10), ("V", 11),
                               ("K", 1, 4), ("K", 1, 5), ("Q", 0, 3),
                               ("Q", 1, 3)]):
            place(1, 1, n, *u)
        for n, u in enumerate([("K", 0, 6), ("V", 12), ("K", 0, 7), ("V", 13),
                               ("V", 14), ("V", 15), ("K", 1, 6),
                               ("K", 1, 7)]):
            place(2, 0, n, *u)
        for cdx in range(4):
            place(2, 0, 8 + cdx, "O", 0, cdx)        # g(0,*) landed ~60us ago
            place(2, 1, 4 + cdx, "O", 0, 4 + cdx)
            place(2, 1, 8 + cdx, "O", 1, cdx)
            place(3, 0, 4 + cdx, "O", 1, 4 + cdx)
            place(3, 0, 8 + cdx, "O", 2, cdx)
            place(3, 1, 4 + cdx, "O", 2, 4 + cdx)

        def run_unit(unit):
            kind = unit[0]
            if kind == "K":
                emit_K(unit[1], unit[2])
            elif kind == "V":
                emit_V(unit[1])
            elif kind == "Q":
                emit_Q(unit[1], unit[2])
            elif kind == "O":
                oproj_unit(unit[1], unit[2])

        # lead-in: Q for j=0, K/V for ktiles 0-3 (hp=0 side)
        emit_Q(0, 0)
        emit_Q(1, 0)
        emit_K(0, 0)
        emit_V(0)
        emit_V(1)
        emit_K(0, 1)
        emit_V(2)
        emit_V(3)

        emit_S(0)
        for k in range(len(steps)):
            if k + 1 < len(steps):
                emit_S(k + 1)
            for unit in fillers[k]:
                run_unit(unit)
            emit_EZ(k)

        # tail: only j=3's output projection remains
        for cdx in range(MC):
            oproj_unit(3, cdx)

        if dbg:
            nc.sync.dma_start(dbg_qt[:], qt_sb[:])
            nc.sync.dma_start(dbg_kt[:], ktz_sb[:])
            nc.sync.dma_start(dbg_v[:], v_sb[:])

    nc.compile()
    return nc


def _prep_inputs(x, W_Q, W_K, W_V, W_O, b_Q, b_K, b_V, b_O, mask):
    x = np.asarray(x, dtype=np.float32)
    W_Q = np.asarray(W_Q, dtype=np.float32)
    W_K = np.asarray(W_K, dtype=np.float32)
    W_V = np.asarray(W_V, dtype=np.float32)
    W_O = np.asarray(W_O, dtype=np.float32)
    b_Q = np.asarray(b_Q, dtype=np.float32)
    b_K = np.asarray(b_K, dtype=np.float32)
    b_O = np.asarray(b_O, dtype=np.float32)
    b_V = np.asarray(b_V, dtype=np.float32)
    mask = np.asarray(mask)

    # effective output bias: b_O + sum_h W_O[h] @ b_V[h]
    bo_eff = b_O + np.einsum("hnd,hd->n", W_O.astype(np.float64),
                             b_V.astype(np.float64)).astype(np.float32)
    # diagonal 128x128 block of the mask, transposed to (k, q); the kernel
    # skips all fully-masked blocks assuming causal structure
    triu = np.ascontiguousarray(mask[0:128, 0:128].T.astype(np.float32))
    # W^T packs: [m, h*64+d]
    wqT = np.ascontiguousarray(W_Q.transpose(2, 0, 1).reshape(D, H * DH))
    wkT = np.ascontiguousarray(W_K.transpose(2, 0, 1).reshape(D, H * DH))
    wvT = np.ascontiguousarray(W_V.transpose(2, 0, 1).reshape(D, H * DH))
    woT = np.ascontiguousarray(W_O.transpose(0, 2, 1).reshape(H * DH, D))

    in_maps = []
    for c in range(NCORES):
        b = c // 4
        g = c % 4
        hs = slice(4 * g * DH, 4 * (g + 1) * DH)
        bqk = np.stack([
            np.concatenate([b_Q[4 * g], b_Q[4 * g + 1]]),
            np.concatenate([b_Q[4 * g + 2], b_Q[4 * g + 3]]),
            np.concatenate([b_K[4 * g], b_K[4 * g + 1]]),
            np.concatenate([b_K[4 * g + 2], b_K[4 * g + 3]]),
        ], axis=1)
        in_maps.append({
            "xT": np.ascontiguousarray(x[b].T).astype(ml_dtypes.bfloat16),
            "wq": np.ascontiguousarray(wqT[:, hs]).astype(ml_dtypes.bfloat16),
            "wk": np.ascontiguousarray(wkT[:, hs]).astype(ml_dtypes.bfloat16),
            "wv": np.ascontiguousarray(wvT[:, hs]).astype(ml_dtypes.bfloat16),
            "wo": np.ascontiguousarray(
                woT[:, NSL * g:NSL * (g + 1)]).astype(ml_dtypes.bfloat16),
            "bqk": np.ascontiguousarray(bqk.astype(np.float32)),
            "bo": np.ascontiguousarray(
                bo_eff[NSL * g:NSL * (g + 1)].reshape(2, 128).T),
            "triu": triu.astype(ml_dtypes.bfloat16),
        })
    return in_maps


last_exec_time_ns = None


def kernel(x, W_Q, W_K, W_V, W_O, b_Q, b_K, b_V, b_O, mask):
    global last_exec_time_ns
    in_maps = _prep_inputs(x, W_Q, W_K, W_V, W_O, b_Q, b_K, b_V, b_O, mask)
    dbg = os.environ.get("KERNEL_DEBUG") == "1"
    if "nc" not in _cache:
        _cache["nc"] = _build(dbg)
    nc = _cache["nc"]

    trace = os.environ.get("KERNEL_TRACE") == "1"
    if trace:
        import sys, types
        import trn_agent_boot.trn_boot as _tb
        hook = _tb._ntff_profile_via_ctypes('/opt/axon/libaxon_pjrt.so')
        mod = types.ModuleType("antenv.axon_hooks")
        mod.get_axon_ntff_profile_hook = lambda: hook
        mod.set_axon_ntff_profile_hook = lambda h: None
        sys.modules["antenv.axon_hooks"] = mod
        bass_utils.upload_artifacts = lambda tmpdir: f"local:{tmpdir}"

    res = bass_utils.run_bass_kernel_spmd(
        nc, in_maps, core_ids=list(range(NCORES)), trace=trace)
    last_exec_time_ns = res.exec_time_ns
    _cache["last_res"] = res

    out = np.empty((B, S, D), dtype=np.float32)
    for c in range(NCORES):
        b = c // 4
        g = c % 4
        out[b, :, NSL * g:NSL * (g + 1)] = res.results[c]["outT"].T
    return out
